# revision 16
# baseline (speedup 1.0000x reference)
"""Trainium2 Bass kernel for a single causal attention head.

Problem: x [8, 2048, 1024] f32, Wq/Wk/Wv [1024, 64] f32 ->
         out [8, 2048, 64] f32  (causal softmax(q k^T / 8) v per batch)

Strategy: data-parallel over batch -- core b computes batch element b,
no collectives. Per core, a column-pipelined flash-style kernel over
4 t-slices of 512.

Key mechanisms (v5):
  * x and W are cast to bf16 on the host (the kernel used bf16 compute
    anyway), halving HBM traffic; every DMA goes through HWDGE.
  * W is the first Sync-queue DMA, then chunked x loads (j0 in 4, j1 in
    2), so the first projection matmul starts ~10us in.
  * ~6 dummy matmuls on a zeroed tile run during the initial DMA wait so
    the PE HAM clock-gate is at 8/8 when real matmuls start.
  * score matmuls (K=64, half the PE rows) run as row-tiled pairs:
    tile A in rows 0-63 (kT/qT at partitions 0-63), tile B in rows
    64-127 (kT native in the qk stack, qT in a partition-swapped copy
    qk2 = [kT;qT]).  The swap is ONE PE matmul against a host-baked
    128x128 permutation matrix -- a SBUF->SBUF DMA here costs ~2.5us of
    completion latency per column and stalls the whole score stream.
  * the causal diagonal mask is folded into the score accumulation: a
    second matmul accumulates a host-baked -1e4 strict-lower-triangle
    onto the diagonal 128x128 window, so exp underflows to 0 there and
    no DVE masking sits between exp and PV.
  * exp is batched: one ACTIVATE per score pair reads [128, 2, 512] f32
    across two PSUM banks; the steep staircase pair of each column only
    computes [256:512].
  * PV windows of column j-1 are interleaved between score pairs of
    column j so TensorE stays dense while ScalarE drains exps; each
    output window is stored as soon as it is normalized.

Layouts (bf16 compute, f32 PSUM accumulation):
  xp     [4, 128, 8, 512] bf16 DRAM (host-marshalled x^T slices)
  wp     [128, 8, 3, 64] bf16 (host-marshalled [Wq|Wk|Wv] chunks)
  qT|kT  [128, T] (q rows 0-63, k rows 64-127), one M=128 matmul chain
  qk2    [kT; qT] partition-swap of qk (for the row-tiled score pair)
  scores [s-chunk 128, t 512] pairs into [128, 2, 512] PSUM groups
  exp    bf16 [128, 2, 512] tiles; causal staircase via block skipping
         + one [128,128] lower-triangle mask multiply on the diagonal
  PV     out[t 128, 65] = expT_block^T @ [v | 1]; column 64 gives the
         softmax denominator; normalize via reciprocal + tensor_scalar.
  out    [4, 128, 4, 64] f32 DRAM, host reassembles to [T, H].
"""

import sys
from contextlib import ExitStack

sys.path.insert(0, "/opt/trn_rl_repo")

import numpy as np
import ml_dtypes

import concourse.bass as bass
import concourse.tile as tile
from concourse import bacc, mybir
from concourse.bass_utils import run_bass_kernel_spmd

B, T, E, H = 8, 2048, 1024, 64
NCORES = 8
TJ = 512            # t-slice width (score tile free dim)
NJ = T // TJ        # 4 columns
NWARM = 12          # dummy matmuls to lift the HAM clock gate to 8/8
BF16 = mybir.dt.bfloat16
F32 = mybir.dt.float32


def build_kernel(tc: "tile.TileContext", out: bass.AP, xp_dram: bass.AP,
                 wp_dram: bass.AP):
    nc = tc.nc
    EXP = mybir.ActivationFunctionType.Exp
    MUL = mybir.AluOpType.mult

    ctx = ExitStack()
    const = ctx.enter_context(tc.tile_pool(name="const", bufs=1))
    expp = ctx.enter_context(tc.tile_pool(name="expp", bufs=20))
    outp = ctx.enter_context(tc.tile_pool(name="outp", bufs=8))
    small = ctx.enter_context(tc.tile_pool(name="small", bufs=8))
    ps_qk = ctx.enter_context(tc.tile_pool(name="ps_qk", bufs=1, space="PSUM"))
    ps_v = ctx.enter_context(tc.tile_pool(name="ps_v", bufs=1, space="PSUM"))
    ps_s = ctx.enter_context(tc.tile_pool(name="ps_s", bufs=2, space="PSUM"))
    ps_o = ctx.enter_context(tc.tile_pool(name="ps_o", bufs=2, space="PSUM"))

    # PE warm-up: zero tile via DVE (first DVE op, ~5us in), then NWARM
    # dummy matmuls that run during the w/x DMA wait.  By the time real
    # data lands the HAM window has seen ~3.5us of PE activity -> 2.4 GHz.
    warm = const.tile([128, TJ], BF16, tag="warm")
    nc.gpsimd.memset(warm[:], 0.0)
    warm_ps = ps_s.tile([128, 2, TJ], F32, tag="s", name="warm_ps")
    for k in range(NWARM):
        nc.tensor.matmul(warm_ps[:, 0, :], warm[:, 0:128], warm[:],
                         start=True, stop=True)

    # Weights + constants in ONE first-position Sync DMA: the first
    # projection matmul is gated on W, and the perm/mask constants gate the
    # first score pair -- both must land before the x stream, and a late
    # constant makes the static scheduler push the whole score stream back.
    wc_sb = const.tile([128, 1920], BF16, tag="wc")
    nc.sync.dma_start(wc_sb[:], wp_dram[:])
    # flat views: W chunk ec lives at cols [ec*192, ec*192+192) as [Wq|Wk|Wv]
    wqk = [wc_sb[:, ec * 192:ec * 192 + 128] for ec in range(8)]
    wv = [wc_sb[:, ec * 192 + 128:ec * 192 + 192] for ec in range(8)]
    perm = wc_sb[:, 1536:1664]
    iden = wc_sb[:, 1664:1792]
    masku = wc_sb[:, 1792:1920]

    # x slices: all four stay resident (32 KB/partition total).  j0 lands
    # in 4 chunks of 2 e-chunks so the first projection matmuls can start
    # as soon as ~256 KB are in; later slices use 2 chunks of 4.
    x_tiles = []
    for j in range(NJ):
        x_tiles.append(const.tile([128, 8, TJ], BF16, tag=f"x{j}", name=f"x{j}"))

    def emit_x_load(j):
        # j0 is on the critical path and its qk chain is paced by chunk
        # completion; descending chunk sizes [4,2,1,1] make the last-needed
        # e-chunks land earliest (receipt latency pipelines with later data)
        bounds = [0, 4, 6, 7, 8] if j == 0 else [0, 4, 8]
        for a, b in zip(bounds, bounds[1:]):
            nc.sync.dma_start(
                x_tiles[j][:, a:b, :], xp_dram[j][:, a:b, :]
            )

    for j in range(NJ):
        emit_x_load(j)


    qk_tiles = []   # per column: [128, TJ] bf16, rows 0:64 = qT, 64:128 = kT
    qk2_tiles = []  # per column: [128, TJ] bf16, rows 0:64 = kT, 64:128 = qT
    v_tiles = []    # per column: [128, 4, H+1] bf16 ([v | ones])
    for j in range(NJ):
        qk_tiles.append(const.tile([128, TJ], BF16, tag=f"qk{j}", name=f"qk{j}"))
        qk2_tiles.append(const.tile([128, TJ], BF16, tag=f"qk2{j}", name=f"qk2{j}"))
        v_tiles.append(const.tile([128, 4, H + 1], BF16, tag=f"v{j}", name=f"v{j}"))

    etiles = {}     # (j, b) -> exp pair tile [128, 2, TJ]

    def emit_proj(j):
        """q/k/v projections for column j (x slice j loads were pre-issued).

        The PSUM->SBUF cast of the q/k stack is split into halves so the
        qk2 partition-swap DMAs (emitted by the caller right after this)
        unblock as early as possible."""
        qk_j, qk2_j, v_j = qk_tiles[j], qk2_tiles[j], v_tiles[j]
        xsl = x_tiles[j]

        # combined q/k projection: psum[0:64]=qT, [64:128]=kT
        psq = ps_qk.tile([128, TJ], F32, tag="qk", name=f"psq{j}")
        for ec in range(8):
            nc.tensor.matmul(
                psq[:], wqk[ec], xsl[:, ec, :],
                start=(ec == 0), stop=(ec == 7),
            )
        nc.vector.tensor_copy(qk_j[:], psq[:])

        # partition swap on the PE: qk2 = P.T @ qk = [kT; qT]
        psq2 = ps_qk.tile([128, TJ], F32, tag="qk", name=f"psq2_{j}")
        nc.tensor.matmul(psq2[:], perm, qk_j[:], start=True, stop=True)
        nc.vector.tensor_copy(qk2_j[:], psq2[:])

        # v projection for s-chunks 4j..4j+3 into one PSUM bank
        nc.vector.memset(v_j[:, :, H], 1.0)
        psv = ps_v.tile([128, 4, H], F32, tag="v", name=f"psv{j}")
        for c in range(4):
            for ec in range(8):
                nc.tensor.matmul(
                    psv[:, c, :], xsl[:, ec, c * 128:(c + 1) * 128],
                    wv[ec], start=(ec == 0), stop=(ec == 7),
                )
        nc.vector.tensor_copy(v_j[:, :, 0:H], psv[:])

    def emit_score_pair(j, b):
        """score tiles i=2b, 2b+1 of column j as a row-tiled pair + one exp."""
        grp = ps_s.tile([128, 2, TJ], F32, tag="s", name=f"pss{j}_{b}")
        f0s = []
        for slot in range(2):
            i = 2 * b + slot
            r = i - 4 * j          # r >= 0 -> staircase block
            f0 = 128 * r if r > 0 else 0
            f0s.append(f0)
            isl = slice((i % 4) * 128, (i % 4 + 1) * 128)
            if slot == 0:
                # PE rows 0-63: kT from the swapped copy, qT native
                nc.tensor.matmul(
                    grp[:, 0, f0:], qk2_tiles[i // 4][0:64, isl],
                    qk_tiles[j][0:64, f0:], start=True, stop=(r < 0),
                    tile_position=(0, 0),
                )
            else:
                # PE rows 64-127: kT native, qT from the swapped copy
                nc.tensor.matmul(
                    grp[:, 1, f0:], qk_tiles[i // 4][64:128, isl],
                    qk2_tiles[j][64:128, f0:], start=True, stop=(r < 0),
                    tile_position=(64, 0),
                )
        # fold the causal mask into the accumulation: the diagonal 128-wide
        # window gets -1e4 above the diagonal, so exp underflows to +0 there
        # with no DVE pass afterwards.  Mask matmuls come after both score
        # matmuls so the row-tiled pair still runs concurrently.
        for slot in range(2):
            r = 2 * b + slot - 4 * j
            if r >= 0:
                f0 = 128 * r if r > 0 else 0
                nc.tensor.matmul(
                    grp[:, slot, f0:f0 + 128], iden, masku,
                    start=False, stop=True,
                )
        e = expp.tile([128, 2, TJ], BF16, tag="e", name=f"e{j}_{b}")
        # batched exp across both PSUM banks; the steep staircase pair
        # (f0 >= 256 in both slots) only needs the [256:] columns.
        t0 = 256 if min(f0s) >= 256 else 0
        nc.scalar.activation(e[:, :, t0:], grp[:, :, t0:], EXP, scale=0.125)
        etiles[(j, b)] = e

    def emit_pv_mms(j, c):
        m = 4 * j + c
        po = ps_o.tile([128, H + 1], F32, tag="po", name=f"po{j}_{c}")
        for i in range(m + 1):
            nc.tensor.matmul(
                po[:], etiles[(j, i // 2)][:, i % 2, c * 128:(c + 1) * 128],
                v_tiles[i // 4][:, i % 4, :],
                start=(i == 0), stop=(i == m),
            )
        return po

    def emit_pv_finish(j, c, po, engine=None):
        rec = small.tile([128, 1], F32, tag="rec", name=f"rec{j}_{c}")
        nc.vector.reciprocal(rec[:], po[:, H:H + 1])
        osb = outp.tile([128, H], F32, tag="o", name=f"osb{j}_{c}")
        nc.vector.tensor_scalar_mul(osb[:], po[:, 0:H], rec[:])
        (engine or nc.sync).dma_start(out[j][c], osb[:])

    def emit_pv_window(j, c):
        """PV + normalize + store for output window c of column j."""
        emit_pv_finish(j, c, emit_pv_mms(j, c))

    # Emission order == per-engine execution order.  Projections are
    # hoisted ahead of the score pairs they don't depend on, so TensorE
    # has dense work while the qk2 swap DMAs are in flight; PV of column
    # j-1 fills TensorE while ScalarE drains column j's exps.
    # All projections and score pairs are emitted first: emission order is
    # the static scheduler's priority, and these feed ScalarE (the
    # co-bottleneck), so they must outrank every PV matmul.  The PV windows
    # follow in chronological order at the lowest priority -- the scheduler
    # slots their matmuls into the exp-WAR stalls of the score stream, which
    # spreads them across the kernel on its own.
    for j in range(NJ):
        emit_proj(j)
        for b in range(2 * j + 2):
            emit_score_pair(j, b)
    for j in range(NJ - 1):
        for c in range(4):
            emit_pv_window(j, c)
    emit_pv_window(NJ - 1, 0)
    emit_pv_window(NJ - 1, 1)
    po2 = emit_pv_mms(NJ - 1, 2)
    po3 = emit_pv_mms(NJ - 1, 3)
    emit_pv_finish(NJ - 1, 2, po2)
    # last store via ScalarE HWDGE (idle by now) so the two completion
    # receipts overlap instead of queueing on Sync
    emit_pv_finish(NJ - 1, 3, po3, engine=nc.scalar)

    ctx.close()


_NC_CACHE = None


def build_nc():
    global _NC_CACHE
    if _NC_CACHE is not None:
        return _NC_CACHE
    nc = bacc.Bacc(
        "TRN2", target_bir_lowering=False, debug=False,
        enable_asserts=False, num_devices=NCORES,
    )
    xp_dram = nc.dram_tensor("xp", [NJ, 128, 8, TJ], BF16, kind="ExternalInput").ap()
    wp_dram = nc.dram_tensor("wp", [128, 1920], BF16, kind="ExternalInput").ap()
    out = nc.dram_tensor("out", [NJ, 4, 128, H], F32, kind="ExternalOutput").ap()
    with tile.TileContext(nc) as tc:
        build_kernel(tc, out, xp_dram, wp_dram)
    nc.finalize()
    _NC_CACHE = nc
    return nc


def _const_cst():
    p = np.arange(128)
    perm = np.zeros((128, 128), dtype=np.float32)
    perm[p, (p + 64) % 128] = 1.0
    iden = np.eye(128, dtype=np.float32)
    masku = np.where(p[:, None] > p[None, :], -1e4, 0.0).astype(np.float32)
    return np.ascontiguousarray(
        np.stack([perm, iden, masku], axis=1)
    ).astype(ml_dtypes.bfloat16)


def _marshal(x_b: np.ndarray):
    # xp[j, p, ec, t'] = x[j*TJ + t', ec*128 + p], cast bf16
    return np.ascontiguousarray(
        x_b.reshape(NJ, TJ, 8, 128).transpose(0, 3, 2, 1)
    ).astype(ml_dtypes.bfloat16)


def _install_profile_hook():
    """The agent image lacks ``antenv.axon_hooks``; inject a shim so
    run_bass_kernel_spmd(trace=True) can reach the axon NTFF profiler."""
    import types

    if "antenv.axon_hooks" not in sys.modules:
        mod = types.ModuleType("antenv.axon_hooks")
        holder = {}
        mod.set_axon_ntff_profile_hook = lambda h: holder.__setitem__("h", h)
        mod.get_axon_ntff_profile_hook = lambda: holder.get("h")
        sys.modules["antenv.axon_hooks"] = mod
    from trn_agent_boot.trn_boot import _ntff_profile_via_ctypes

    hook = _ntff_profile_via_ctypes("/opt/axon/libaxon_pjrt.so")
    sys.modules["antenv.axon_hooks"].set_axon_ntff_profile_hook(hook)
    # no fish bucket in this container -- keep artifacts local
    from concourse import bass_utils as bu

    bu.upload_artifacts = lambda tmpdir: tmpdir


def run(inputs: dict, trace: bool = False, tmpdir: str | None = None):
    """Returns (out [8, 2048, 64] f32, exec_time_ns or None)."""
    x = np.asarray(inputs["x"], dtype=np.float32)
    # wp[p, ec, r, h] = W_r[ec*128 + p, h], cast bf16
    wqkv = np.stack([np.asarray(inputs["Wq"]), np.asarray(inputs["Wk"]),
                     np.asarray(inputs["Wv"])]).astype(np.float32)
    w_pre = np.ascontiguousarray(
        wqkv.reshape(3, 8, 128, H).transpose(2, 1, 0, 3)
    ).astype(ml_dtypes.bfloat16)
    nc = build_nc()
    if trace:
        _install_profile_hook()
    wc = np.concatenate(
        [w_pre.reshape(128, 1536), _const_cst().reshape(128, 384)], axis=1
    )
    in_maps = [{"xp": _marshal(x[b]), "wp": wc} for b in range(B)]
    res = run_bass_kernel_spmd(
        nc, in_maps, core_ids=list(range(NCORES)), trace=trace, tmpdir=tmpdir
    )
    # out[j, c, p, h]: (j, c, p) lexicographic == t = j*512 + c*128 + p
    out = np.stack([
        res.results[b]["out"].reshape(T, H) for b in range(B)
    ]).astype(np.float32)
    return out, res.exec_time_ns


def kernel(**inputs) -> np.ndarray:
    out, _ = run(inputs)
    return out


if __name__ == "__main__":
    rng = np.random.default_rng(0)
    ins = {
        "x": rng.standard_normal((B, T, E), dtype=np.float32),
        "Wq": rng.uniform(-1 / 32, 1 / 32, (E, H)).astype(np.float32),
        "Wk": rng.uniform(-1 / 32, 1 / 32, (E, H)).astype(np.float32),
        "Wv": rng.uniform(-1 / 32, 1 / 32, (E, H)).astype(np.float32),
    }
    o, ns = run(ins, trace=False)
    print("out", o.shape, o.dtype, "exec_ns", ns)
